# revision 1
# baseline (speedup 1.0000x reference)
"""Multi-head self-attention (B=2, N=2048, D=1024, H=16) on 8 Trainium2 cores.

Sharding: core c -> batch b = c // 4, head group g = c % 4 (heads 4g..4g+3,
as two pairs).  The attention path runs in fp8 (e4m3 operands, e5m2 softmax
weights) using DoubleRow matmuls (two fp8 k-tiles per pass = 0.5 cycles/row);
accuracy survives because the residual `x` carries ~94% of the output norm
and the host combines partial projections in float64.

Score pre-conditioning is computed by the PE itself:
  sc = 5.770780 * (q.k / 8) + 32     [q scaled 0.7213475 at evacuation;
                                      +32 via an augmented q/k row 8*4]
so the softmax weight e^z / 128 (z = q.k / 8) is EXACTLY the e5m2 bitcast of
round(clamp(sc, 0, 123)) (Schraudolph).  Each score half-tile [128, 512]
goes to either engine: DVE does the one-op clamp-convert, ACT a true Exp
with matching scale/bias; a greedy balancer equalizes engine load.  The
per-row normalization cancels the /128; tiny PWL mismatch is diluted ~16x.

Layouts: scores use two independent psum half-tile rings (scA/scB) so the
score matmul for tile t+2 only waits on the exp of tile t in its own ring —
this breaks the exp->matmul->exp latency loop that otherwise leaves both
engines ~35% idle.  kT/qT live in a [33, 2]-slot DoubleRow layout built by
SBUF->SBUF DMA shuffles; PV accumulates [v | 1]^T e per jt-pair (row 64 =
denominator); reciprocal + 64x broadcast matmul + DVE multiply produce the
fp8 attnT; the projection contracts all 256 head dims in one DoubleRow pass
per output tile and streams bf16 partials to DRAM per i-tile.
Host divides by 4096 (64 attn scale * 64 wp scale) and adds x in float64.
"""

import numpy as np
import ml_dtypes

import concourse.bass as bass
import concourse.bacc as bacc
import concourse.mybir as mybir
import concourse.tile as tile
from concourse.bass_utils import run_bass_kernel_spmd

B = 2
N = 2048
D = 1024
NH = 16
DH = 64
N_CORES = 8
TP = 4                 # head-parallel ways per batch
HPC = NH // TP         # 4 heads per core
HDIM = HPC * DH        # 256 head dims per core
PAIRS = 2

IT = 4                 # i-tiles of 512
JT = 16                # j-chunks of 128

SCH_A = 5.770780163555851      # 4*log2(e) * 8 ... b = SCH_A*z + 32
QSCALE = SCH_A / 8.0           # applied to q at evacuation
ACT_SCALE = 1.0 / SCH_A * 8.0 / 8.0    # 1/5.77078
ACT_BIAS = -32.0 / SCH_A - float(np.log(128.0))

F32 = mybir.dt.float32
F32R = mybir.dt.float32r
BF16 = mybir.dt.bfloat16
F8 = mybir.dt.float8e4
F8E5 = mybir.dt.float8e5
U8 = mybir.dt.uint8
AF = mybir.ActivationFunctionType
DR = mybir.MatmulPerfMode.DoubleRow
ALU = mybir.AluOpType

E4NP = ml_dtypes.float8_e4m3


class Balancer:
    """Greedy ACT/DVE load balancer for elementwise psum-evacuation ops."""

    def __init__(self):
        self.t = {"act": 0.0, "dve": 0.0}

    bias = 10000.0

    def pick(self, cost_act, cost_dve):
        if self.t["act"] + cost_act + self.bias <= self.t["dve"] + cost_dve:
            self.t["act"] += cost_act
            return "act"
        self.t["dve"] += cost_dve
        return "dve"

    def force(self, eng, cost):
        self.t[eng] += cost


def build_bass():
    nc = bacc.Bacc("TRN2", target_bir_lowering=False, debug=False)
    x_d = nc.declare_dram_parameter("xdr", [128, 4, 2, N], F8, isOutput=False)
    wq_d = nc.declare_dram_parameter("wq", [128, 4, 2, HDIM], F8, isOutput=False)
    wk_d = nc.declare_dram_parameter("wk", [128, 4, 2, HDIM], F8, isOutput=False)
    wv_d = nc.declare_dram_parameter("wv", [128, 4, 2, HDIM], F8, isOutput=False)
    wp_d = nc.declare_dram_parameter("wp", [128, 2, D], F8, isOutput=False)
    aq_d = nc.declare_dram_parameter("aug_q", [2, 2, 2, N], F8, isOutput=False)
    ak_d = nc.declare_dram_parameter("aug_k", [2, 2, 2, N], F8, isOutput=False)
    sel_d = nc.declare_dram_parameter("sel64", [1, 2, 128], F32R, isOutput=False)
    pT_d = nc.declare_dram_parameter("pT", [D, N], BF16, isOutput=True)

    bal = Balancer()

    with tile.TileContext(nc) as tc:
        with (
            tc.tile_pool(name="big", bufs=1) as big,
            tc.tile_pool(name="stage", bufs=2) as stage,
            tc.tile_pool(name="exps", bufs=2) as exps,
            tc.tile_pool(name="psum", bufs=1, space="PSUM") as psum,
        ):
            # ---- constants / inputs (order: gate-first) ----
            ws = {}
            for nm, src in (("k", wk_d), ("q", wq_d), ("v", wv_d)):
                t = big.tile([128, 4, 2, HDIM], F8, tag=f"w{nm}")
                eng = nc.scalar if nm in ("k", "q") else nc.sync
                eng.dma_start(out=t, in_=src[:, :, :, :])
                ws[nm] = t
            xs = big.tile([128, 4, 2, N], F8, tag="xs")
            xq = {(0, 0): nc.sync, (2, 0): nc.sync, (3, 0): nc.sync,
                  (0, 1): nc.scalar, (2, 1): nc.scalar, (1, 1): nc.scalar,
                  (1, 0): nc.gpsimd, (3, 1): nc.gpsimd}
            for c, s in ((0, 0), (0, 1), (1, 0), (2, 0), (2, 1), (3, 0),
                         (1, 1), (3, 1)):
                xq[(c, s)].dma_start(out=xs[:, c, s, :], in_=x_d[:, c, s, :])
            sel64 = big.tile([1, 2, 128], F32R, tag="sel64")
            nc.sync.dma_start(out=sel64, in_=sel_d[0:1, :, :])
            recip_pad = big.tile([1, 1024], F32R, tag="recip_pad")
            wps = big.tile([128, 2, D], F8, tag="wp")
            nc.sync.dma_start(out=wps, in_=wp_d[:, :, :])

            bias_t = big.tile([128, 1], F32, tag="bias")
            nc.vector.memset(bias_t, ACT_BIAS)

            # qT/kT in scores-DR layout: [part 33h+r, pair, slot, tok]
            qT = big.tile([97, 2, 2, N], F8, tag="qT")
            kT = big.tile([97, 2, 2, N], F8, tag="kT")
            # aug rows (row 32 of each head block; slot0 = 8 (q) / 4 (k),
            # slot1 = 0) come from tiny DRAM consts to keep engines free.
            def emit_aug(p_):
                for hh in range(2):
                    r = 64 * hh + 32
                    qeng = nc.gpsimd if p_ == 0 else nc.sync
                    keng = nc.scalar if p_ == 0 else nc.gpsimd
                    qeng.dma_start(out=qT[r:r + 1, p_, :, :],
                                   in_=aq_d[hh:hh + 1, p_, :, :])
                    keng.dma_start(out=kT[r:r + 1, p_, :, :],
                                   in_=ak_d[hh:hh + 1, p_, :, :])

            emit_aug(0)

            # v with trailing ones column: [tok, jt, head(4), 65] (pad 80)
            v8 = big.tile([128, JT, 4, 80], F8, tag="v8")
            nc.gpsimd.memset(v8[:, :, :, 64:65], 1.0)

            attnT = big.tile([128, 2, N], F8, tag="attnT")

            # act-table warm-up (Exp table also serves Copy)
            warm = big.tile([1, 1], F32, tag="warm")
            nc.scalar.activation(warm, bias_t[0:1, 0:1], AF.Exp)

            C_EVAC1K = (1024 + 222) * 0.833 + 57
            C_EVAC1K_D = (1024 + 120) * 1.042 + 70
            C_EXP = C_EVAC1K
            C_EXP_D = C_EVAC1K_D
            C_512 = (512 + 222) * 0.833 + 57
            C_512_D = (512 + 120) * 1.042 + 70

            def evac(out_ap, in_ap, scale, cost_a, cost_d, force=None):
                if force is None:
                    eng = bal.pick(cost_a, cost_d)
                else:
                    eng = force
                    bal.force(eng, cost_a if eng == "act" else cost_d)
                if eng == "act":
                    nc.scalar.activation(out_ap, in_ap, AF.Copy, scale=scale)
                elif scale != 1.0:
                    nc.vector.tensor_scalar_mul(out_ap, in_ap, scale)
                else:
                    nc.vector.tensor_copy(out_ap, in_ap)

            # ---- P1 emitters (interleaved into P2 via gen_q) ----
            def emit_qk(nm, dstT, p):
                w_s = ws[nm]
                st2 = stage.tile([128, 2, 1024], F8, tag="qk_st", bufs=3)
                for itp in range(2):  # it-pairs -> 1024 tokens each
                    for ii in range(2):
                        ps = psum.tile([128, 512], F32,
                                       tag=("scA" if ii == 0 else "scB"),
                                       bufs=2)
                        tok = itp * 1024 + ii * 512
                        for ci, c in enumerate((0, 2, 3, 1)):
                            nc.tensor.matmul(
                                ps,
                                lhsT=w_s[:, c, :, p * 128:(p + 1) * 128],
                                rhs=xs[:, c, :, tok:tok + 512],
                                start=(ci == 0),
                                stop=(ci == 3),
                                perf_mode=DR,
                            )
                        evac(st2[:, itp, ii * 512:(ii + 1) * 512], ps,
                             QSCALE if nm == "q" else 1.0, C_512, C_512_D,
                             force=(("act" if ii == 0 else "dve")
                                    if p == 0 else None))
                for hh in range(2):
                    for s in range(2):
                        r = 64 * hh + 32 * s
                        deng = nc.sync if hh == 0 else nc.gpsimd
                        deng.dma_start(
                            out=dstT[64 * hh:64 * hh + 32, p, s, :],
                            in_=st2[r:r + 32, :, :],
                        )

            def emit_v(tp):
                ps = psum.tile([128, 512], F32, tag="vps", bufs=2)
                for jj in range(2):
                    t = 2 * tp + jj
                    for c in range(4):
                        nc.tensor.matmul(
                            ps[:, jj * 256:(jj + 1) * 256].rearrange(
                                "p (h d) -> p h d", d=64),
                            lhsT=xs[:, c, :, t * 128:(t + 1) * 128],
                            rhs=ws["v"][:, c, :, :],
                            start=(c == 0),
                            stop=(c == 3),
                            perf_mode=DR,
                        )
                evac(
                    v8[:, 2 * tp:2 * tp + 2, :, 0:64],
                    ps.rearrange("p (j h d) -> p j h d", h=4, d=64),
                    1.0, C_512, C_512_D,
                )

            # minimal prefix: everything P2(p0) needs; pair-1 q/k are
            # generated while pair-0 attention runs.
            emit_qk("k", kT, 0)
            emit_qk("q", qT, 0)
            emit_aug(1)
            for tp in range(JT // 2):
                emit_v(tp)
            gen_q = [("k", kT, 1), ("q", qT, 1)]

            def emit_scores(p, it, jt):
                scs = []
                for hh in range(2):
                    sch = psum.tile([128, 512], F32,
                                    tag=("scA" if hh == 0 else "scB"), bufs=2)
                    nc.tensor.matmul(
                        sch,
                        lhsT=kT[64 * hh:64 * hh + 33, p, :,
                                jt * 128:(jt + 1) * 128],
                        rhs=qT[64 * hh:64 * hh + 33, p, :,
                               it * 512:(it + 1) * 512],
                        start=True,
                        stop=True,
                        perf_mode=DR,
                    )
                    scs.append(sch)
                return scs

            for p in range(PAIRS):
                for it in range(IT):
                    if gen_q and (p == 0) and (it >= 1):
                        nm_, dst_, p_ = gen_q.pop(0)
                        emit_qk(nm_, dst_, p_)
                    e8 = exps.tile([128, JT, 2, 512], U8, tag="e8", bufs=3)
                    pv2 = psum.tile([65, 2, 512], F32, tag="pv2", bufs=1)
                    pvA = pv2[:, 0, :]
                    pvB = pv2[:, 1, :]
                    sc_next = emit_scores(p, it, 0)
                    for jt in range(JT):
                        sc_pair = sc_next
                        if jt + 1 < JT:
                            sc_next = emit_scores(p, it, jt + 1)
                        for hh in range(2):
                            if (jt == JT - 1 or (p == 0 and it <= 1)
                                    or (p == 1 and it == IT - 1)):
                                eng = "act" if hh == 0 else "dve"
                                bal.force(eng, C_512 if hh == 0 else C_512_D)
                            else:
                                eng = bal.pick(C_512, C_512_D)
                            e_out = e8[:, jt, hh, :]
                            sc_h = sc_pair[hh]
                            if eng == "act":
                                nc.scalar.activation(
                                    e_out.bitcast(F8E5), sc_h, AF.Exp,
                                    bias=bias_t, scale=ACT_SCALE,
                                )
                            else:
                                nc.vector.tensor_scalar(
                                    e_out, sc_h, 123.0, 0.0,
                                    ALU.min, ALU.max,
                                )
                        if jt % 2 == 1:
                            st, sp = (jt == 1), (jt == JT - 1)
                            for hh, pvx in ((0, pvA), (1, pvB)):
                                nc.tensor.matmul(
                                    pvx,
                                    lhsT=v8[:, jt - 1:jt + 1, 2 * p + hh, 0:65],
                                    rhs=e8[:, jt - 1:jt + 1, hh, :].bitcast(F8E5),
                                    start=st,
                                    stop=sp,
                                    perf_mode=DR,
                                )
                    last = (p == 1 and it == IT - 1)
                    with nc.allow_low_precision(reason="f8 softmax denom"):
                        if last:
                            nc.vector.reciprocal(recip_pad[:, 0:512],
                                                 pv2[64:65, 0, :])
                            nc.vector.reciprocal(recip_pad[:, 512:1024],
                                                 pv2[64:65, 1, :])
                            bal.force("dve", 2 * C_512_D)
                        else:
                            nc.vector.reciprocal(recip_pad[0:1, :],
                                                 pv2[64:65, :, :])
                            bal.force("dve", C_EVAC1K_D)
                    bc = psum.tile([128, 512], F32, tag="vps", bufs=2)
                    nc.tensor.matmul(bc, lhsT=sel64[:, 0, :],
                                     rhs=recip_pad[:, 0:512],
                                     start=True, stop=False)
                    nc.tensor.matmul(bc, lhsT=sel64[:, 1, :],
                                     rhs=recip_pad[:, 512:1024],
                                     start=False, stop=True)
                    bc_sb = stage.tile([128, 512], F32R, tag="bc_sb", bufs=3)
                    evac(bc_sb, bc, 1.0, C_512, C_512_D)
                    nc.vector.tensor_tensor(
                        attnT[0:64, p, it * 512:(it + 1) * 512],
                        pvA[0:64, :], bc_sb[0:64, :], ALU.mult)
                    nc.vector.tensor_tensor(
                        attnT[64:128, p, it * 512:(it + 1) * 512],
                        pvB[0:64, :], bc_sb[64:128, :], ALU.mult)
                    bal.force("dve", 2 * C_512_D)

                    if p == 1:
                        # ---- P3 (streamed): projection for this i-tile ----
                        for og in range(D // 256):
                            st = stage.tile([128, 2, 512], BF16, tag="pj_st", bufs=4)
                            for oo in range(2):
                                ot = 2 * og + oo
                                pj = psum.tile([128, 512], F32, tag="vps",
                                               bufs=2)
                                nc.tensor.matmul(
                                    pj,
                                    lhsT=wps[:, :, ot * 128:(ot + 1) * 128],
                                    rhs=attnT[:, :, it * 512:(it + 1) * 512],
                                    start=True,
                                    stop=True,
                                    perf_mode=DR,
                                )
                                evac(st[:, oo, :], pj, 1.0, C_512, C_512_D)
                            if p == 1 and it == IT - 1:
                                for oo in range(2):
                                    eng = nc.sync if oo == 0 else nc.scalar
                                    eng.dma_start(
                                        out=pT_d[(2 * og + oo) * 128:
                                                 (2 * og + oo + 1) * 128,
                                                 it * 512:(it + 1) * 512],
                                        in_=st[:, oo, :],
                                    )
                            else:
                                nc.sync.dma_start(
                                    out=pT_d[og * 256:(og + 1) * 256,
                                             it * 512:(it + 1) * 512].rearrange(
                                                 "(a b) n -> b a n", a=2),
                                    in_=st,
                                )
    return nc


_NC = None


def _get_nc():
    global _NC
    if _NC is None:
        _NC = build_bass()
        _NC.finalize()
    return _NC


_AUG_Q = np.zeros((2, 2, 2, N), np.float32)
_AUG_Q[:, :, 0, :] = 8.0
_AUG_K = np.zeros((2, 2, 2, N), np.float32)
_AUG_K[:, :, 0, :] = 4.0
_AUG_Q = _AUG_Q.astype(E4NP)
_AUG_K = _AUG_K.astype(E4NP)



def _dr4(w):  # [1024, 256] -> [128, 4, 2, 256] fp8
    return np.ascontiguousarray(
        w.reshape(4, 2, 128, -1).transpose(2, 0, 1, 3).astype(E4NP))


_SEL64 = np.zeros((1, 2, 128), np.float32)
_SEL64[0, 0, 0:64] = 64.0
_SEL64[0, 1, 64:128] = 64.0


def make_in_maps(x, w_qkv, w_proj):
    x = np.asarray(x, np.float32)
    w_qkv = np.asarray(w_qkv, np.float32)
    w_proj = np.asarray(w_proj, np.float32)
    xdrs = []
    for b in range(B):
        xT = np.ascontiguousarray(x[b].T).astype(E4NP)  # [D, N]
        xdrs.append(np.ascontiguousarray(
            xT.reshape(4, 2, 128, N).transpose(2, 0, 1, 3)))
    in_maps = []
    for c in range(N_CORES):
        b, g = divmod(c, TP)
        h0 = g * HDIM
        wp = (64.0 * w_proj[h0:h0 + HDIM, :]).astype(E4NP)  # [256, 1024]
        in_maps.append({
            "xdr": xdrs[b],
            "wq": _dr4(w_qkv[:, h0:h0 + HDIM]),
            "wk": _dr4(w_qkv[:, D + h0:D + h0 + HDIM]),
            "wv": _dr4(w_qkv[:, 2 * D + h0:2 * D + h0 + HDIM]),
            "wp": np.ascontiguousarray(
                wp.reshape(2, 128, D).transpose(1, 0, 2)),
            "sel64": _SEL64,
            "aug_q": _AUG_Q,
            "aug_k": _AUG_K,
        })
    return in_maps


def combine_outputs(x, results):
    x = np.asarray(x, np.float32)
    out = np.empty((B, N, D), np.float32)
    for b in range(B):
        acc = x[b].astype(np.float64)
        for g in range(TP):
            pT = np.asarray(results[b * TP + g]["pT"]).astype(np.float64)
            acc += pT.T / 4096.0
        out[b] = acc.astype(np.float32)
    return out


def kernel(x, w_qkv, w_proj):
    nc = _get_nc()
    in_maps = make_in_maps(x, w_qkv, w_proj)
    res = run_bass_kernel_spmd(nc, in_maps, list(range(N_CORES))).results
    return combine_outputs(x, res)

